# revision 18
# baseline (speedup 1.0000x reference)
"""Trainium2 Bass kernel for a GNN message-passing layer (BoundaryConvLayer).

Computation (reference, per node i over D=128 channels):
    rate  = softplus(x @ W_rate) + EPS
    gamma = x @ W_rob + b_rob
    h     = x @ W_fc + b_fc
    agg   = segment_sum(h[row] + h[col], row)
    y     = LayerNorm((rate*agg + gamma) / (1 + rate*deg + EPS)) * ln_gamma + ln_beta

Distribution: nodes sharded across 8 cores by contiguous row blocks; edges
partitioned by destination row so the segment sum is local.  Every core
computes the full (bias-free) GEMM g = x @ W_fc and stores it in its own DRAM
copy as the gather table; per-edge h[col] rows are then fetched locally with
the batched DMAGatherAnt instruction.

Key identities used:
    agg[i] = cnt[i]*(g[i] + 2*b_fc) + sum_{e:row=i} g[col_e]
  where cnt = in-edge count (h = g + b_fc).  The neighbor sum is a one-hot
  "selection matrix" matmul on the PE over gathered edge rows; the self+bias
  term is a diag(cnt) matmul against gl2 = g_local + 2*b_fc, with g_local
  recomputed on the PE (the program is SPMD-identical across cores, so a
  per-core DRAM offset read-back is not expressible).

Gather layout: dma_gather indices are int16, so the g table is split into
NCHK chunks of CPAD (< 32768) rows; nodes are remapped so each chunk holds
RPC real rows followed by zero rows (zero because the padded x columns are
zero), which serve as padding targets for unused grid slots.  Per node tile
and chunk there are Cq 128-slot groups; each (chunk, tile-group) pair is one
dense dma_gather instruction.  Within each (tile, chunk) run the slots are
sorted by source column so the random HBM reads are ascending.
"""

import numpy as np
import ml_dtypes
from contextlib import ExitStack
from dataclasses import dataclass

import concourse.bass as bass
import concourse.tile as tile
from concourse import bacc, mybir
from concourse.bass_utils import run_bass_kernel_spmd

# The stock ACT-table chooser greedily picks the first set containing each
# function, which for {Exp, Ln, Copy, Square} alternates between two sets and
# reloads the table ~150x per run (~1.3us each).  Restrict it to the one set
# that contains all four so a single load suffices.
_ACT_KEEP = "natural_log_exp_and_others"
if not getattr(bacc, "_act_tables_patched", False):
    _orig_get_tables = bacc.get_activation_tables

    def _patched_get_tables(arch):
        t = _orig_get_tables(arch)
        if _ACT_KEEP in t:
            t = {k: (v if k == _ACT_KEEP else set()) for k, v in t.items()}
        return t

    bacc.get_activation_tables = _patched_get_tables
    bacc._act_tables_patched = True

BF16 = ml_dtypes.bfloat16
EPS = 1e-4
LN_EPS = 1e-5
P = 128
D = 128


@dataclass
class Cfg:
    N: int            # total nodes
    E: int            # total edges
    NC: int           # cores
    NCHK: int = 4     # gather table chunks (int16 range)
    Cq: int = 0       # 128-slot groups per (tile, chunk); set by prep
    ln_trivial: bool = False

    @property
    def NLOC(self):
        return self.N // self.NC

    @property
    def T(self):
        return (self.NLOC + P - 1) // P

    @property
    def TLP(self):
        return self.T * P

    @property
    def RPC(self):    # real nodes per chunk
        return self.N // self.NCHK

    @property
    def CPAD(self):   # padded chunk rows (>=64 zero rows, 128-aligned)
        return ((self.RPC + 64 + P - 1) // P) * P

    @property
    def NPAD(self):   # g table rows
        return self.NCHK * self.CPAD

    @property
    def G(self):      # tiles per gather group
        for g in (7, 14, 4, 2, 1):
            if self.T % g == 0:
                return g
        return 1


def prep(x, edge_index, degree, W_fc, b_fc, W_rate, W_rob, b_rob, ln_gamma, ln_beta,
         cfg: Cfg):
    """Host-side preprocessing: shard + build per-core gather/selection tables."""
    N, NC, NCHK = cfg.N, cfg.NC, cfg.NCHK
    NLOC, T, TLP = cfg.NLOC, cfg.T, cfg.TLP
    RPC, CPAD, NPAD = cfg.RPC, cfg.CPAD, cfg.NPAD
    assert N % NCHK == 0 and NC % NCHK == 0 and CPAD <= 32767
    CPC = NC // NCHK  # cores per chunk
    # last core of each chunk must fit its padded tile range inside the chunk
    assert (CPC - 1) * NLOC + TLP <= CPAD

    x = np.asarray(x, np.float32)
    edge_index = np.asarray(edge_index, np.int64)
    degree = np.asarray(degree)
    row, col = edge_index[0], edge_index[1]

    xT = np.zeros((P, NPAD), BF16)
    xTf = x.T.astype(BF16)
    for q in range(NCHK):
        xT[:, q * CPAD:q * CPAD + RPC] = xTf[:, q * RPC:(q + 1) * RPC]

    w_fc = np.ascontiguousarray(W_fc, dtype=np.float32).astype(BF16)
    # fused rate|gamma weights [128, 2D]
    w_rg = np.concatenate([
        np.ascontiguousarray(W_rate, dtype=np.float32).astype(BF16),
        np.ascontiguousarray(W_rob, dtype=np.float32).astype(BF16)], axis=1)
    # 2*b_fc broadcast over rows (added to local g before the diag matmul)
    bfc2 = np.broadcast_to((2.0 * np.asarray(b_fc, np.float32)).astype(BF16)[None, :],
                           (P, D)).copy()
    # b_rob broadcast over rows (added during gamma PSUM->SBUF move)
    brob = np.broadcast_to(np.asarray(b_rob, np.float32)[None, :], (P, D)).copy()
    ident = np.eye(P, dtype=BF16)

    cfg.ln_trivial = bool(np.all(np.asarray(ln_gamma) == 1.0)
                          and np.all(np.asarray(ln_beta) == 0.0))
    lnab = np.zeros((P, 2 * D), np.float32)
    lnab[:, :D] = np.asarray(ln_gamma, np.float32)[None, :]
    lnab[:, D:] = np.asarray(ln_beta, np.float32)[None, :]

    core_of = row // NLOC
    chunk_of_col = col // RPC
    KW = CPAD // P      # wrapped-table blocks per partition
    # chunk-local logical row -> wrapped DRAM row (partition-major layout)
    cr_all = col % RPC
    s_all = (cr_all % P) * KW + cr_all // P
    PAD_S = (RPC % P) * KW + RPC // P    # a guaranteed zero row

    # pass 1: per-core, per-tile, per-chunk edge counts fix the global Cq
    percore = []
    maxslots = 0
    for r in range(NC):
        m = core_of == r
        rl = row[m] - r * NLOC
        ce = s_all[m]
        cq = chunk_of_col[m]
        cnt = np.bincount(rl, minlength=TLP)
        tq = (rl // P) * NCHK + cq
        cnt_tq = np.bincount(tq, minlength=T * NCHK).reshape(T, NCHK)
        maxslots = max(maxslots, int(cnt_tq.max()))
        percore.append((rl, ce, cq, cnt, cnt_tq))
    Cq = max(1, -(-maxslots // P))
    cfg.Cq = Cq
    NG = T // cfg.G
    IPG = cfg.G * Cq * P       # idxs per (chunk, group) instruction

    in_maps = []
    for r in range(NC):
        rl, ce, cq, cnt, cnt_tq = percore[r]
        # order edges by (tile, chunk, col): ascending source columns within
        # each (tile, chunk) run so the gather's random HBM reads ascend
        order = np.lexsort((ce, cq, rl // P))
        rl_s, ce_s, cq_s = rl[order], ce[order], cq[order]
        t_s = rl_s // P
        tq_s = t_s * NCHK + cq_s
        run_start = np.zeros(T * NCHK + 1, np.int64)
        np.cumsum(cnt_tq.reshape(-1), out=run_start[1:])
        pos = np.arange(len(rl_s)) - run_start[tq_s]
        # idx-stream position within instruction (q, gg):
        #   msg layout [P, tile-in-group, q, c, D]; stream for chunk q covers
        #   slots [tl, c, p] in that order -> i = tl*(Cq*128) + c*128 + p
        tl_s = t_s % cfg.G
        gg_s = t_s // cfg.G
        ipos = tl_s * (Cq * P) + pos
        # pad slots are multiplied by zero in the sel matmul, so their
        # content is irrelevant: forward-fill each pad with the last real
        # index in stream order -- a duplicate fetch is row-buffer hot and
        # keeps the ascending bank pattern intact (the old shared zero-row
        # target broke every streak)
        idx16 = np.full((NCHK, NG, IPG), -1, np.int16)
        idx16[cq_s, gg_s, ipos] = ce_s.astype(np.int16)
        flat = idx16.reshape(NCHK * NG, IPG)
        mask = flat >= 0
        ffidx = np.where(mask, np.arange(IPG)[None, :], 0)
        np.maximum.accumulate(ffidx, axis=1, out=ffidx)
        flat = flat[np.arange(flat.shape[0])[:, None], ffidx]
        flat[flat < 0] = PAD_S          # leading pads before any real index
        idx16 = flat.reshape(NCHK, NG, IPG)
        # wrap each stream: idx i -> [i%16, i//16], replicate to 128 partitions
        idxw = idx16.reshape(NCHK, NG, IPG // 16, 16).transpose(0, 1, 3, 2)
        idxw = np.ascontiguousarray(idxw)
        idxw = np.tile(idxw, (1, 1, 8, 1))           # [NCHK, NG, 128, IPG//16]
        idx_sb = np.ascontiguousarray(
            idxw.transpose(2, 0, 1, 3)).reshape(P, NCHK * NG * (IPG // 16))

        # rowsr: rebased row (node % 128) per slot, -1 for pads
        rowsr = np.full((P, T * NCHK * Cq), -1.0, BF16)
        slot_col = t_s * (NCHK * Cq) + cq_s * Cq + pos // P
        rowsr[pos % P, slot_col] = (rl_s % P).astype(BF16)

        iotab = np.broadcast_to(
            np.tile(np.arange(P, dtype=BF16)[None, :], (1, NCHK * Cq)),
            (P, NCHK * Cq * P)).copy()

        cntb = cnt.astype(np.float32).reshape(T, P).T.astype(BF16)
        degl = np.zeros(TLP, np.float32)
        degl[:NLOC] = degree[r * NLOC:(r + 1) * NLOC].astype(np.float32)
        degf = degl.reshape(T, P).T.copy()
        g0 = (r * NLOC // RPC) * CPAD + (r * NLOC % RPC)
        xTloc = np.ascontiguousarray(xT[:, g0:g0 + TLP])

        in_maps.append({
            "xT": xT, "xTloc": xTloc,
            "Wfc": w_fc, "Wrg": w_rg,
            "bfc2": bfc2, "brob": brob, "ident": ident, "lnab": lnab,
            "iotab": iotab, "rowsr": rowsr, "idxs": idx_sb,
            "cntb": cntb, "degf": degf,
        })
    return in_maps


def build(cfg: Cfg, g0_by_core=None):
    """Build the SPMD Bass program (identical on every core)."""
    NC, T, TLP, NPAD = cfg.NC, cfg.T, cfg.TLP, cfg.NPAD
    NCHK, Cq, CPAD = cfg.NCHK, cfg.Cq, cfg.CPAD
    RPC, NLOC = cfg.RPC, cfg.NLOC
    G = cfg.G
    NG = T // G
    IPG = G * Cq * P
    SELW = NCHK * Cq * P       # sel width per tile
    bf = mybir.dt.bfloat16
    f32 = mybir.dt.float32
    i16 = mybir.dt.int16

    nc = bacc.Bacc("TRN2", target_bir_lowering=False, debug=False, num_devices=NC,
                   num_swdge_queues=4)
    for cv in (LN_EPS, EPS, 1.0 + EPS, float(D) * float(D) * LN_EPS,
               float(np.log(D))):
        cs = nc.alloc_sbuf_tensor(f"const-float32-{cv}", [P, 1], f32)
        nc.gpsimd.memset(cs.ap(), cv)
        nc.const_aps.aps[(f32, cv)] = cs.ap()
    nc.all_engine_barrier()

    d_xT = nc.dram_tensor("xT", [P, NPAD], bf, kind="ExternalInput").ap()
    d_xTloc = nc.dram_tensor("xTloc", [P, TLP], bf, kind="ExternalInput").ap()
    d_wfc = nc.dram_tensor("Wfc", [P, D], bf, kind="ExternalInput").ap()
    d_wrg = nc.dram_tensor("Wrg", [P, 2 * D], bf, kind="ExternalInput").ap()
    d_bfc2 = nc.dram_tensor("bfc2", [P, D], bf, kind="ExternalInput").ap()
    d_brob = nc.dram_tensor("brob", [P, D], f32, kind="ExternalInput").ap()
    d_ident = nc.dram_tensor("ident", [P, P], bf, kind="ExternalInput").ap()
    d_lnab = nc.dram_tensor("lnab", [P, 2 * D], f32, kind="ExternalInput").ap()
    d_iota = nc.dram_tensor("iotab", [P, SELW], bf, kind="ExternalInput").ap()
    d_rowsr = nc.dram_tensor("rowsr", [P, T * NCHK * Cq], bf,
                             kind="ExternalInput").ap()
    d_idxs = nc.dram_tensor("idxs", [P, NCHK * NG * (IPG // 16)], i16,
                            kind="ExternalInput").ap()
    d_cntb = nc.dram_tensor("cntb", [P, T], bf, kind="ExternalInput").ap()
    d_degf = nc.dram_tensor("degf", [P, T], f32, kind="ExternalInput").ap()
    # one g-table tensor per chunk so chunk-q gathers depend only on chunk-q
    # phase-1 writes; partition-wrapped layout [P, (CPAD//P)*D] so the writes
    # are contiguous per partition (the gather indices are pre-wrapped)
    KW = CPAD // P
    d_gq = [nc.dram_tensor(f"gtab{q}", [P, KW * D], bf, kind="Internal").ap()
            for q in range(NCHK)]
    d_y = nc.dram_tensor("y", [TLP, D], f32, kind="ExternalOutput").ap()

    with tile.TileContext(nc) as tc, ExitStack() as ctx:
        from concourse import library_config
        nc.gpsimd.load_library(library_config.mlp)
        consts = ctx.enter_context(tc.tile_pool(name="consts", bufs=1))
        wfc = consts.tile([P, D], bf)
        nc.sync.dma_start(wfc[:], d_wfc[:])

        # phase-3 consts loaded up front so early gathers aren't queued
        # behind all of phase 1 on the sync DMA ring
        wrg = consts.tile([P, 2 * D], bf)
        nc.sync.dma_start(wrg[:], d_wrg[:])
        bfc2 = consts.tile([P, D], bf)
        nc.sync.dma_start(bfc2[:], d_bfc2[:])
        brob = consts.tile([P, D], f32)
        nc.sync.dma_start(brob[:], d_brob[:])
        ident = consts.tile([P, P], bf)
        nc.sync.dma_start(ident[:], d_ident[:])
        iota = consts.tile([P, SELW], bf)
        nc.sync.dma_start(iota[:], d_iota[:])
        rowsr = consts.tile([P, T * NCHK * Cq], bf)
        nc.sync.dma_start(rowsr[:], d_rowsr[:])
        idxs = consts.tile([P, NCHK * NG * (IPG // 16)], i16)
        cntb = consts.tile([P, T], bf)
        nc.sync.dma_start(cntb[:], d_cntb[:])
        degf = consts.tile([P, T], f32)
        nc.sync.dma_start(degf[:], d_degf[:])
        xloc = consts.tile([P, TLP], bf)
        nc.sync.dma_start(xloc[:], d_xTloc[:])
        lnab = None
        if not cfg.ln_trivial:
            lnab = consts.tile([P, 2 * D], f32)
            nc.sync.dma_start(lnab[:], d_lnab[:])


        # ---------------- phase 1: g = x @ W_fc for all nodes ----------------
        CHUNK = 8192
        GRP = 512
        cast_engines = [nc.vector, nc.scalar, nc.vector]
        ci = 0
        def gather_group(msg, gg, qs):
            for q in qs:
                icol = (q * NG + gg) * (IPG // 16)
                sec = msg[:, q * G * Cq * D:(q + 1) * G * Cq * D]
                nc.gpsimd.dma_gather(
                    out_ap=sec.rearrange("p (s d) -> p s d", d=D),
                    in_ap=d_gq[q].rearrange("p (t d) -> (p t) d", d=D),
                    idxs_ap=idxs[:, icol:icol + IPG // 16],
                    num_idxs=IPG,
                    num_idxs_reg=IPG,
                    elem_size=D,
                    single_packet=False,
                    queue_num=q % 4,
                )
        with tc.tile_pool(name="p1x", bufs=3) as p1x, \
             tc.tile_pool(name="p1ps", bufs=6, space="PSUM") as p1ps, \
             tc.tile_pool(name="p1st", bufs=3) as p1st:
            for q in range(NCHK):
                for c0 in range(0, CPAD, CHUNK):
                    cw = min(CHUNK, CPAD - c0)
                    xc = p1x.tile([P, CHUNK], bf, tag="xc", name="xc")
                    nc.sync.dma_start(xc[:, :cw],
                                      d_xT[:, q * CPAD + c0:q * CPAD + c0 + cw])
                    gst = p1st.tile([P, CHUNK], bf, tag="gst", name="gst")
                    for g0 in range(0, cw, GRP):
                        gw = min(GRP, cw - g0)
                        gps = p1ps.tile([P, GRP], f32, space="PSUM", tag="gps",
                                        name="gps")
                        for j in range(0, gw, P):
                            nc.tensor.matmul(
                                out=gps[:, j:j + P],
                                lhsT=xc[:, g0 + j:g0 + j + P],
                                rhs=wfc[:],
                                start=True, stop=True,
                            )
                        eng = cast_engines[ci % 3]
                        ci += 1
                        if eng is nc.scalar:
                            eng.copy(gst[:, g0:g0 + gw], gps[:, :gw])
                        else:
                            # 2-port DVE op, but GpSimd is idle in phase 1
                            eng.tensor_scalar_mul(out=gst[:, g0:g0 + gw],
                                                  in0=gps[:, :gw], scalar1=1.0)
                    # batched table write on the ACT HWDGE ring so reads
                    # (sync ring) and writes stream in parallel; wrapped
                    # layout makes this a plain contiguous 2D copy
                    nc.scalar.dma_start(
                        d_gq[q][:, (c0 // P) * D:((c0 + cw) // P) * D],
                        gst[:, :cw])

        # ---------------- phase 3: message passing + elementwise -------------
        selp = ctx.enter_context(tc.tile_pool(name="selp", bufs=3))
        aggps = ctx.enter_context(tc.tile_pool(name="aggps", bufs=4, space="PSUM"))
        ratps = ctx.enter_context(tc.tile_pool(name="ratps", bufs=2, space="PSUM"))
        glps = ctx.enter_context(tc.tile_pool(name="glps", bufs=2, space="PSUM"))
        eltp = ctx.enter_context(tc.tile_pool(name="eltp", bufs=2))
        smallp = ctx.enter_context(tc.tile_pool(name="smallp", bufs=2))
        B = 4

        def eltwise(bt, tiles, veng=None):
            veng = veng or nc.vector
            nb = len(tiles)
            rate4, agg4, gam4 = bt
            r3 = rate4[:, :nb, :]
            a3 = agg4[:, :nb, :]
            g3 = gam4[:, :nb, :]
            num = eltp.tile([P, B, D], f32, tag="num", name="num")[:, :nb, :]
            den = eltp.tile([P, B, D], f32, tag="den", name="den")[:, :nb, :]
            y0 = eltp.tile([P, B, D], f32, tag="y0", name="y0")[:, :nb, :]
            sq = eltp.tile([P, B, D], f32, tag="sq", name="sq")[:, :nb, :]
            yf = eltp.tile([P, B, D], f32, tag="yf", name="yf")
            st = smallp.tile([P, 8 * B], f32, tag="st", name="st")
            s1 = st[:, 0:nb]
            s2 = st[:, B:B + nb]
            mean = st[:, 2 * B:2 * B + nb]
            msq = st[:, 3 * B:3 * B + nb]
            var = st[:, 4 * B:4 * B + nb]
            rstd = st[:, 5 * B:5 * B + nb]

            # DVE ops restricted to tensor_tensor/tensor_reduce/reciprocal
            # (never grab the GpSimd-shared SBUF port pair, so SWDGE
            # descriptor generation for the gathers is never blocked);
            # constant adds/scales run on ACT instead.
            veng.tensor_tensor(out=num, in0=r3, in1=a3,
                               op=mybir.AluOpType.mult)
            veng.tensor_add(out=num, in0=num, in1=g3)
            t0g = tiles[0]
            degb = degf[:, t0g:t0g + nb][:, :, None].to_broadcast([P, nb, D])
            veng.tensor_tensor(out=den, in0=r3, in1=degb,
                               op=mybir.AluOpType.mult)
            # 1/den = exp(-ln(den)) on ACT: the DVE RECIPROCAL instruction
            # measures ~3.3us per batch, the two ACT table ops ~0.9us
            nc.scalar.activation(out=den, in_=den,
                                 func=mybir.ActivationFunctionType.Ln,
                                 bias=1.0 + EPS)
            nc.scalar.activation(out=den, in_=den,
                                 func=mybir.ActivationFunctionType.Exp,
                                 scale=-1.0)
            veng.tensor_mul(out=y0, in0=num, in1=den)
            veng.tensor_tensor(out=sq, in0=y0, in1=y0,
                              op=mybir.AluOpType.mult)
            nc.vector.tensor_reduce(out=s1, in_=y0, axis=mybir.AxisListType.X,
                                    op=mybir.AluOpType.add)
            nc.vector.tensor_reduce(out=s2, in_=sq, axis=mybir.AxisListType.X,
                                    op=mybir.AluOpType.add)
            # var*D^2 = D*s2 - s1^2; rstd = (var+eps)^-0.5
            #         = exp(-0.5*ln(D*s2 - s1^2 + D^2*eps) + ln(D))
            nc.vector.tensor_tensor(out=msq, in0=s1, in1=s1,
                                    op=mybir.AluOpType.mult)
            nc.scalar.mul(var, s2, float(D))
            nc.vector.tensor_sub(out=var, in0=var, in1=msq)
            nc.scalar.mul(mean, s1, 1.0 / D)
            nc.scalar.activation(out=var, in_=var,
                                 func=mybir.ActivationFunctionType.Ln,
                                 bias=float(D) * float(D) * LN_EPS)
            nc.scalar.activation(out=rstd, in_=var,
                                 func=mybir.ActivationFunctionType.Exp,
                                 scale=-0.5, bias=float(np.log(D)))
            meanb = mean[:, :, None].to_broadcast([P, nb, D])
            rstdb = rstd[:, :, None].to_broadcast([P, nb, D])
            yf3 = yf[:, :nb, :]
            veng.tensor_sub(out=yf3, in0=y0, in1=meanb)
            veng.tensor_mul(out=yf3, in0=yf3, in1=rstdb)
            if lnab is not None:
                lg = lnab[:, 0:D][:, None, :].to_broadcast([P, nb, D])
                lb = lnab[:, D:2 * D][:, None, :].to_broadcast([P, nb, D])
                nc.vector.tensor_mul(out=yf3, in0=yf3, in1=lg)
                nc.vector.tensor_add(out=yf3, in0=yf3, in1=lb)
            n0 = tiles[0] * P
            nw = nb * P
            dst = d_y[n0:n0 + nw, :].rearrange("(t p) d -> p t d", p=P)
            nc.sync.dma_start(dst, yf[:, :nb, :])

        # idxs lands on the ACT ring after every table write (FIFO per
        # ring), so no gather can start before phase 1 fully drains -- the
        # random gather reads must not share HBM with the sequential phase
        nc.scalar.dma_start(idxs[:], d_idxs[:])

        msgp = ctx.enter_context(tc.tile_pool(name="msgp", bufs=4))
        bt = None
        for gg in range(NG):
            tg0 = gg * G
            # msg layout: [P, q, tile-in-group, c, D] -- q outermost so each
            # chunk's gather writes one contiguous [P, G*Cq, D] section
            msg = msgp.tile([P, NCHK * G * Cq * D], bf, tag="msg",
                            name="msg")
            gather_group(msg, gg, range(NCHK))
            for tl in range(G):
                t = tg0 + tl
                j = t % B
                if j == 0:
                    bt = (eltp.tile([P, B, D], f32, tag="rate4", name="rate4"),
                          eltp.tile([P, B, D], f32, tag="agg4", name="agg4"),
                          eltp.tile([P, B, D], f32, tag="gam4", name="gam4"))
                sel = selp.tile([P, SELW], bf, tag="sel", name="sel")
                rb = rowsr[:, t * NCHK * Cq:(t + 1) * NCHK * Cq][:, :, None] \
                    .to_broadcast([P, NCHK * Cq, P])
                nc.vector.tensor_tensor(
                    out=sel.rearrange("p (c m) -> p c m", c=NCHK * Cq),
                    in0=iota.rearrange("p (c m) -> p c m", c=NCHK * Cq),
                    in1=rb,
                    op=mybir.AluOpType.is_equal)
                diag = selp.tile([P, P], bf, tag="diag", name="diag")
                nc.vector.tensor_tensor(
                    out=diag[:], in0=ident[:],
                    in1=cntb[:, t:t + 1].to_broadcast([P, P]),
                    op=mybir.AluOpType.mult)
                # local g rows for the self term: recompute on the PE, then
                # add 2*b_fc so the diag(cnt) matmul carries the bias term too
                glp = glps.tile([P, D], f32, space="PSUM", tag="glp", name="glp")
                nc.tensor.matmul(out=glp[:], lhsT=xloc[:, t * P:(t + 1) * P],
                                 rhs=wfc[:], start=True, stop=True)
                gl = selp.tile([P, D], bf, tag="gl", name="gl")
                nc.vector.tensor_tensor(out=gl[:], in0=glp[:], in1=bfc2[:],
                                        op=mybir.AluOpType.add)
                aps = aggps.tile([P, D], f32, space="PSUM", tag="aps", name="aps")
                nc.tensor.matmul(out=aps[:], lhsT=diag[:], rhs=gl[:],
                                 start=True, stop=False)
                for q in range(NCHK):
                    for c in range(Cq):
                        cc = q * Cq + c
                        moff = ((q * G + tl) * Cq + c) * D
                        last = (q == NCHK - 1) and (c == Cq - 1)
                        nc.tensor.matmul(
                            out=aps[:], lhsT=sel[:, cc * P:(cc + 1) * P],
                            rhs=msg[:, moff:moff + D],
                            start=False, stop=last)
                # fused rate|gamma GEMM: one LDWEIGHTS, 256-wide stream
                rps = ratps.tile([P, 2 * D], f32, space="PSUM", tag="rps",
                                 name="rps")
                nc.tensor.matmul(out=rps[:], lhsT=xloc[:, t * P:(t + 1) * P],
                                 rhs=wrg[:], start=True, stop=True)
                # softplus(z) = ln(exp(z)+1): one ACT table (exp/ln) throughout
                spt = selp.tile([P, D], f32, tag="spt", name="spt")
                nc.scalar.activation(out=spt[:], in_=rps[:, 0:D],
                                     func=mybir.ActivationFunctionType.Exp)
                nc.scalar.activation(out=bt[0][:, j, :], in_=spt[:],
                                     func=mybir.ActivationFunctionType.Ln,
                                     bias=1.0)
                nc.scalar.add(bt[0][:, j, :], bt[0][:, j, :], EPS)
                nc.scalar.copy(bt[1][:, j, :], aps[:])
                # gamma = x@W_rob + b_rob: bias folded into the PSUM move
                nc.vector.tensor_tensor(out=bt[2][:, j, :], in0=rps[:, D:2 * D],
                                        in1=brob[:], op=mybir.AluOpType.add)
                if j == B - 1 or t == T - 1:
                    # tail groups: desc-gen is done, Pool is idle -- drain
                    # the eltwise backlog on gpsimd (SBUF-only tt ops)
                    tailv = nc.gpsimd if gg >= NG - 2 else None
                    eltwise(bt, list(range(t - j, t + 1)), veng=tailv)

    nc.compile()
    return nc


def run(inputs, cfg: Cfg, core_ids=None):
    in_maps = prep(**inputs, cfg=cfg)
    nc = build(cfg)
    cores = core_ids or list(range(cfg.NC))
    # warmup execution: the very first NEFF execution on a freshly booted
    # device has been observed (once) to return corrupted gather results;
    # run twice and keep the second result
    run_bass_kernel_spmd(nc, in_maps, core_ids=cores)
    res = run_bass_kernel_spmd(nc, in_maps, core_ids=cores)
    ys = [res.results[r]["y"][:cfg.NLOC] for r in range(cfg.NC)]
    return np.concatenate(ys, axis=0)


def kernel(**inputs):
    cfg = Cfg(N=100_000, E=800_000, NC=8)
    return run(inputs, cfg)
